# revision 16
# baseline (speedup 1.0000x reference)
"""GAT message-passing kernel for Trainium2 (8 NeuronCores, Bass/Tile).

v2 strategy (edge parallelism, batched SWDGE dma_gather):
  - Host: edges sorted by dst; 8 equal node ranges (12500 dst nodes/core).
    Per core, supertile = 128 contiguous dst nodes, capacity 2560 edge slots
    laid out as 4 src-bank segments (bank = src%4) of 640 slots each; pads
    use dummy index 0 with slot-id PAD (one-hot column of zeros).
  - Phase 1 (replicated): T[n] = feat@fc_w (128 fp16 = 256B rows) and
    ERT (er packed 32 nodes/row, 256B) via one matmul pass over feat.
  - Phase 2 per supertile group (G=4): ONE dma_gather per src bank pulls
    all 4 supertiles' feat_src rows (0.34ns/descriptor on Q7 vs 8.5ns for
    per-chunk indirect DMA); one dma_gather pulls packed er rows by dst//32.
    el is computed on-device (fs . attn_l per head); er selected from the
    packed row by a 32-wide one-hot + tree reduce. ex=exp(leaky_relu(el+er));
    scatter-to-slot aggregation via one-hot matmuls accumulating
    [fs*ex | ex] into a [128 slot, 132] PSUM; epilogue divides by the
    summed ex (segment softmax) and streams 128 node rows out per supertile.
"""

import numpy as np

# ---------------- problem constants (hardcoded; kernel.py is self-contained) ---
N = 100000
F = 128
H = 4
D = 32
HD = H * D          # 128
NEG = 0.2
NCORES = 8

NPAD = 100352       # 784 tiles of 128 node rows
NT = NPAD // 128    # 784
NODES_PC = N // NCORES      # 12500 dst nodes per core
NSUP = 98                   # ceil(12500/128) supertiles with real nodes
G = 4                       # supertiles per gather group
NSUP_P = 100                # padded to a multiple of G
NGRP = NSUP_P // G          # 25
W = 2560                    # edge slots per supertile (20 chunks of 128)
NCH = W // 128              # 20
NBANK = 4                   # src banks (src % 4)
Q = W // NBANK              # 640 slots per (supertile, bank)
QCH = Q // 128              # 5 chunks per bank segment
BROWS = NPAD // NBANK       # 25088 rows per T bank
ERT_ROWS = NPAD // 32       # 3136 packed er rows
PAD_SEG = 999.0
ML = HD + H                 # 132 psum cols

WCH = 2048                  # phase-1 feat columns per DMA
G1 = 8                      # node tiles per T write


def _pack(src, dst):
    """Host-side index preprocessing -> per-core static tables."""
    src = np.asarray(src, np.int64)
    dst = np.asarray(dst, np.int64)
    order = np.argsort(dst, kind="stable")
    s_src = src[order]
    s_dst = dst[order]
    cb = np.searchsorted(s_dst, np.arange(NCORES + 1) * NODES_PC)
    cores = []
    for k in range(NCORES):
        e0, e1 = int(cb[k]), int(cb[k + 1])
        ksrc = s_src[e0:e1]
        kdst = s_dst[e0:e1]
        lo = k * NODES_PC
        sup = (kdst - lo) // 128
        bank = ksrc % NBANK
        o2 = np.lexsort((bank, sup))
        ksrc, kdst, sup, bank = ksrc[o2], kdst[o2], sup[o2], bank[o2]
        sb = sup * NBANK + bank
        cnt = np.bincount(sb, minlength=NSUP_P * NBANK)
        assert cnt.max() <= Q, f"bank quota overflow: {cnt.max()} > {Q}"
        seg_start = np.concatenate(([0], np.cumsum(cnt)))[:-1]
        pos_in_seg = np.arange(len(ksrc)) - seg_start[sb]
        w = bank * Q + pos_in_seg            # slot within supertile [0, W)
        lane = w % 128
        ch = w // 128                        # chunk in [0, NCH)

        segt = np.full((128, NSUP_P * NCH), PAD_SEG, np.float16)
        segt[lane, sup * NCH + ch] = (kdst - lo - sup * 128).astype(np.float16)
        dmt = np.zeros((128, NSUP_P * NCH), np.float16)
        dmt[lane, sup * NCH + ch] = (kdst % 32).astype(np.float16)

        # fs gather indices: call (g, b) covers i_call = (sup%G)*Q + pos_in_seg
        idxf = np.zeros((16, NGRP * NBANK * (G * Q) // 16), np.int16)
        call = (sup // G) * NBANK + bank
        i_call = (sup % G) * Q + pos_in_seg
        idxf[i_call % 16, call * (G * Q // 16) + i_call // 16] = \
            (ksrc // NBANK).astype(np.int16)
        # er gather indices: call g covers j_call = (sup%G)*W + w
        idxe = np.zeros((16, NGRP * (G * W) // 16), np.int16)
        j_call = (sup % G) * W + w
        idxe[j_call % 16, (sup // G) * (G * W // 16) + j_call // 16] = \
            (kdst // 32).astype(np.int16)

        cores.append({
            "idxf": np.ascontiguousarray(np.tile(idxf, (8, 1))),
            "idxe": np.ascontiguousarray(np.tile(idxe, (8, 1))),
            "segt": np.ascontiguousarray(segt),
            "dmt": np.ascontiguousarray(dmt),
        })
    return cores


def _patch_queue_aware_lanes():
    """Make tile's DMASW sem-lane assignment SWDGE-queue-aware.

    Stock TileClockTick rotates the 8 DMASW lanes mod 8 in scheduled order,
    oblivious to queue_num; with num_swdge_queues=4 a lane then serves
    multiple queues (HW sem hazard, CoreSim rejects it). Pin queue q to the
    lane pair {q, q+4} by presetting next_sw_dma_idx before each Pool DMA.
    """
    import concourse.tile_sem_assignment as tsa
    import concourse.mybir as mybir
    from concourse.tile_scheduler import DMAInst
    import concourse.bass_isa as bass_isa

    if getattr(tsa, "_gnn_queue_lanes", False):
        return
    tsa._gnn_queue_lanes = True
    orig = tsa.TileClockTick._assign_tick

    def patched(self, inst):
        if (isinstance(inst, DMAInst)
                and not isinstance(inst, bass_isa.UserSyncedRemoteDMADescs)
                and inst.engine == mybir.EngineType.Pool):
            q = getattr(inst, "queue_num", 0) or 0
            tog = getattr(self, "_gnn_qtog", None)
            if tog is None:
                tog = {}
                self._gnn_qtog = tog
            self.next_sw_dma_idx = q + 4 * tog.get(q, 0)
            tog[q] = 1 - tog.get(q, 0)
        return orig(self, inst)

    tsa.TileClockTick._assign_tick = patched


def _build():
    import concourse.bacc as bacc
    import concourse.tile as tile
    import concourse.mybir as mybir

    _patch_queue_aware_lanes()

    F16 = mybir.dt.float16
    F32 = mybir.dt.float32
    I16 = mybir.dt.int16
    AOT = mybir.AluOpType

    nc = bacc.Bacc("TRN2", target_bir_lowering=False, debug=False,
                   num_swdge_queues=4)
    featT = nc.dram_tensor("featT", [F, NPAD], F16, kind="ExternalInput")
    waug = nc.dram_tensor("waug", [F, ML], F16, kind="ExternalInput")
    wlb = nc.dram_tensor("wlb", [F, HD], F16, kind="ExternalInput")
    consts = nc.dram_tensor("consts", [128, 160], F16, kind="ExternalInput")
    idxf_d = nc.dram_tensor("idxf", [128, NGRP * NBANK * G * Q // 16], I16,
                            kind="ExternalInput")
    idxe_d = nc.dram_tensor("idxe", [128, NGRP * G * W // 16], I16,
                            kind="ExternalInput")
    segt_d = nc.dram_tensor("segt", [128, NSUP_P * NCH], F16,
                            kind="ExternalInput")
    dmt_d = nc.dram_tensor("dmt", [128, NSUP_P * NCH], F16,
                           kind="ExternalInput")
    T = nc.dram_tensor("T", [BROWS, NBANK * 128], F16, kind="Internal")
    ERT = nc.dram_tensor("ERT", [ERT_ROWS, 128], F16, kind="Internal")
    out = nc.dram_tensor("out", [NSUP_P * 128, HD], F32, kind="ExternalOutput")

    with tile.TileContext(nc) as tc:
        with tc.tile_pool(name="const", bufs=1) as const:
            waug_sb = const.tile([F, ML], F16)
            nc.sync.dma_start(out=waug_sb[:], in_=waug[:, :])
            wlb_sb = const.tile([F, HD], F16)
            nc.sync.dma_start(out=wlb_sb[:], in_=wlb[:, :])
            cst = const.tile([128, 160], F16)
            nc.sync.dma_start(out=cst[:], in_=consts[:, :])
            iota128 = cst[:, 0:128]
            iota32 = cst[:, 128:160]
            er_sb = const.tile([128, NT * H], F16)

            # ---- phase 1: T = feat @ fc_w (fp16), ERT = packed er ----
            with tc.tile_pool(name="fp", bufs=3) as fpool, \
                 tc.tile_pool(name="p1ps", bufs=8, space="PSUM") as p1ps, \
                 tc.tile_pool(name="st1", bufs=4) as st1p:
                tpw = WCH // 128
                for wb in range(NPAD // WCH):
                    fsb = fpool.tile([F, WCH], F16)
                    nc.sync.dma_start(
                        out=fsb[:], in_=featT[:, wb * WCH:(wb + 1) * WCH])
                    for grp in range(tpw // G1):
                        stg = st1p.tile([F, G1 * ML], F16)
                        sv = stg[:].rearrange("p (j c) -> p j c", j=G1)
                        for j in range(G1):
                            ps = p1ps.tile([128, ML], F32)
                            col0 = (grp * G1 + j) * 128
                            nc.tensor.matmul(
                                out=ps[:],
                                lhsT=fsb[:, col0:col0 + 128],
                                rhs=waug_sb[:],
                                start=True, stop=True,
                            )
                            nc.vector.tensor_copy(
                                out=sv[:, j, :], in_=ps[:])
                        t0 = wb * tpw + grp * G1
                        nc.vector.tensor_copy(
                            out=er_sb[:].rearrange(
                                "p (t h) -> p t h", h=H)[:, t0:t0 + G1, :],
                            in_=sv[:, :, HD:HD + H])
                        # node n=(t0+j)*128+p -> T row n//4, col (n%4)*128+c
                        nc.sync.dma_start(
                            out=T[t0 * 32:(t0 + G1) * 32, :].rearrange(
                                "(j q) (b c) -> (q b) j c", q=32, c=128),
                            in_=sv[:, :, 0:HD],
                        )
                # node n=t*128+p -> ERT row n//32, col (n%32)*4+h
                nc.sync.dma_start(
                    out=ERT[:, :].rearrange("(t a) (b h) -> (a b) t h",
                                            a=4, h=H),
                    in_=er_sb[:].rearrange("p (t h) -> p t h", h=H),
                )

            # ---- phase 2 ----
            with tc.tile_pool(name="ix", bufs=3) as ixp, \
                 tc.tile_pool(name="sg", bufs=3) as sgp, \
                 tc.tile_pool(name="fsg", bufs=2) as fsgp, \
                 tc.tile_pool(name="erg", bufs=2) as ergp, \
                 tc.tile_pool(name="pt", bufs=2) as ptp, \
                 tc.tile_pool(name="sc", bufs=2) as scp, \
                 tc.tile_pool(name="mx", bufs=4) as mxp, \
                 tc.tile_pool(name="ot", bufs=3) as otp, \
                 tc.tile_pool(name="p2ps", bufs=6, space="PSUM") as p2ps:
                qrr = [0]
                for g in range(NGRP):
                    idxf_sb = ixp.tile([128, NBANK * G * Q // 16], I16, tag="f")
                    nc.sync.dma_start(
                        out=idxf_sb[:],
                        in_=idxf_d[:, g * NBANK * G * Q // 16:
                                   (g + 1) * NBANK * G * Q // 16])
                    idxe_sb = ixp.tile([128, G * W // 16], I16, tag="e")
                    nc.sync.dma_start(
                        out=idxe_sb[:],
                        in_=idxe_d[:, g * G * W // 16:(g + 1) * G * W // 16])
                    seg_sb = sgp.tile([128, G * NCH], F16, tag="s")
                    nc.sync.dma_start(
                        out=seg_sb[:],
                        in_=segt_d[:, g * G * NCH:(g + 1) * G * NCH])
                    dm_sb = sgp.tile([128, G * NCH], F16, tag="d")
                    nc.sync.dma_start(
                        out=dm_sb[:],
                        in_=dmt_d[:, g * G * NCH:(g + 1) * G * NCH])

                    # ucode caps one call at 1024 descriptors; round-robin the
                    # 4 SWDGE queues (desc-gen contexts run concurrently)
                    def _gather(out_ap, in_ap, idxs_ap, nidx, estep):
                        nc.gpsimd.dma_gather(
                            out_ap=out_ap, in_ap=in_ap, idxs_ap=idxs_ap,
                            num_idxs=nidx, num_idxs_reg=nidx,
                            elem_size=128, elem_step=estep,
                            queue_num=qrr[0])
                        qrr[0] = (qrr[0] + 1) % 4

                    fsg = fsgp.tile([128, NBANK * G * Q], F16)
                    for b in range(NBANK):
                        for p0 in range(0, G * Q, 1024):
                            n = min(1024, G * Q - p0)
                            a0 = b * G * Q + p0
                            _gather(
                                fsg[:, a0:a0 + n]
                                    .rearrange("p (k e) -> p k e", e=128),
                                T[:, b * 128:(b + 1) * 128],
                                idxf_sb[:, a0 // 16:(a0 + n) // 16],
                                n, NBANK * 128)
                    erg = ergp.tile([128, G * W], F16)
                    for p0 in range(0, G * W, 1024):
                        n = min(1024, G * W - p0)
                        _gather(
                            erg[:, p0:p0 + n]
                                .rearrange("p (k e) -> p k e", e=128),
                            ERT[:, :],
                            idxe_sb[:, p0 // 16:(p0 + n) // 16],
                            n, 128)

                    for s4 in range(G):
                        sup = g * G + s4
                        # fs view: [128, bank, chunk-in-bank, 128]
                        fs_s = fsg[:].rearrange(
                            "p (b s j e) -> p b s j e", b=NBANK, s=G, e=128
                        )[:, :, s4]
                        seg_s = seg_sb[:, s4 * NCH:(s4 + 1) * NCH]
                        dm_s = dm_sb[:, s4 * NCH:(s4 + 1) * NCH]
                        erv = erg[:].rearrange(
                            "p (s k m h) -> p s k m h", s=G, m=32, h=H
                        )[:, s4]

                        # one-hot edge->slot (P_t) for aggregation
                        ptt = ptp.tile([128, NCH * 128], F16)
                        nc.vector.tensor_tensor(
                            out=ptt[:].rearrange("p (c s) -> p c s", s=128),
                            in0=seg_s[:, :, None].broadcast_to([128, NCH, 128]),
                            in1=iota128[:, None, :].broadcast_to([128, NCH, 128]),
                            op=AOT.is_equal,
                        )

                        # el = fs . attn_l per head
                        tmp = mxp.tile([128, W], F16, tag="tmp")
                        nc.vector.tensor_tensor(
                            out=tmp[:].rearrange("p (b j e) -> p b j e",
                                                 b=NBANK, e=128),
                            in0=fs_s,
                            in1=wlb_sb[:, None, None, :].broadcast_to(
                                [128, NBANK, QCH, 128]),
                            op=AOT.mult,
                        )
                        el = scp.tile([128, NCH * H], F32, tag="el")
                        nc.vector.tensor_reduce(
                            el[:].rearrange("p (c h) -> p c h", h=H),
                            tmp[:].rearrange("p (c h d) -> p c h d", h=H, d=D),
                            mybir.AxisListType.X,
                            AOT.add,
                        )

                        # er select: one-hot over 32-node pack + tree reduce
                        oh = scp.tile([128, NCH * 32], F16, tag="oh")
                        nc.vector.tensor_tensor(
                            out=oh[:].rearrange("p (c m) -> p c m", m=32),
                            in0=dm_s[:, :, None].broadcast_to([128, NCH, 32]),
                            in1=iota32[:, None, :].broadcast_to([128, NCH, 32]),
                            op=AOT.is_equal,
                        )
                        t0 = scp.tile([128, NCH * 128], F16, tag="t0")
                        t0v = t0[:].rearrange("p (c m h) -> p c m h", m=32, h=H)
                        nc.vector.tensor_tensor(
                            out=t0v,
                            in0=erv,
                            in1=oh[:].rearrange("p (c m) -> p c m", m=32)
                                [:, :, :, None].broadcast_to([128, NCH, 32, H]),
                            op=AOT.mult,
                        )
                        m = 32
                        cur = t0v
                        cur_tile = t0
                        while m > 1:
                            m //= 2
                            nxt = scp.tile([128, NCH * m * H], F16,
                                           tag=f"t{m}")
                            nv = nxt[:].rearrange("p (c m h) -> p c m h",
                                                  m=m, h=H)
                            nc.vector.tensor_tensor(
                                out=nv, in0=cur[:, :, 0:m], in1=cur[:, :, m:2 * m],
                                op=AOT.add)
                            cur = nv
                            cur_tile = nxt
                        # u = el + er ; leaky relu ; exp
                        u = scp.tile([128, NCH * H], F32, tag="u")
                        nc.vector.tensor_tensor(
                            out=u[:], in0=el[:], in1=cur_tile[:],
                            op=AOT.add)
                        u2 = scp.tile([128, NCH * H], F32, tag="u2")
                        nc.vector.scalar_tensor_tensor(
                            out=u2[:], in0=u[:], scalar=NEG, in1=u[:],
                            op0=AOT.mult, op1=AOT.max)
                        ex = scp.tile([128, NCH * H], F32, tag="ex")
                        nc.scalar.activation(
                            out=ex[:], in_=u2[:],
                            func=mybir.ActivationFunctionType.Exp)
                        exv = ex[:].rearrange("p (c h) -> p c h", h=H)

                        # mx = [fs * ex | ex]
                        mx = mxp.tile([128, NCH * ML], F16, tag="mx")
                        mv = mx[:].rearrange("p (c w) -> p c w", w=ML)
                        mv4 = mx[:].rearrange("p (b j w) -> p b j w",
                                              b=NBANK, w=ML)
                        ex4 = ex[:].rearrange("p (b j h) -> p b j h",
                                              b=NBANK, h=H)
                        nc.vector.tensor_copy(out=mv[:, :, HD:HD + H], in_=exv)
                        for h in range(H):
                            nc.vector.tensor_tensor(
                                out=mv4[:, :, :, h * D:(h + 1) * D],
                                in0=fs_s[:, :, :, h * D:(h + 1) * D],
                                in1=ex4[:, :, :, h:h + 1].broadcast_to(
                                    [128, NBANK, QCH, D]),
                                op=AOT.mult,
                            )

                        # aggregation
                        ps = p2ps.tile([128, ML], F32)
                        for c in range(NCH):
                            nc.tensor.matmul(
                                out=ps[:],
                                lhsT=ptt[:, c * 128:(c + 1) * 128],
                                rhs=mx[:, c * ML:(c + 1) * ML],
                                start=(c == 0), stop=(c == NCH - 1),
                            )
                        r0 = otp.tile([128, H], F32, tag="r0")
                        nc.vector.tensor_scalar_max(r0[:], ps[:, HD:HD + H],
                                                    1e-30)
                        r1 = otp.tile([128, H], F32, tag="r1")
                        nc.vector.reciprocal(r1[:], r0[:])
                        ot = otp.tile([128, HD], F32, tag="ot")
                        nc.vector.tensor_tensor(
                            out=ot[:].rearrange("p (h d) -> p h d", h=H),
                            in0=ps[:, 0:HD].rearrange("p (h d) -> p h d", h=H),
                            in1=r1[:, :, None].broadcast_to([128, H, D]),
                            op=AOT.mult,
                        )
                        nc.sync.dma_start(
                            out=out[sup * 128:(sup + 1) * 128, :],
                            in_=ot[:],
                        )

    nc.compile()
    return nc


_NC_CACHE = {}
LAST_RESULTS = None


def _get_program():
    if "v2" not in _NC_CACHE:
        _NC_CACHE["v2"] = _build()
    return _NC_CACHE["v2"]


def _host_tables(feat, fc_w, attn_l, attn_r):
    feat = np.asarray(feat, np.float32)
    fc_w = np.asarray(fc_w, np.float64)
    attn_l = np.asarray(attn_l, np.float64).reshape(H, D)
    attn_r = np.asarray(attn_r, np.float64).reshape(H, D)

    featT = np.zeros((F, NPAD), np.float16)
    featT[:, :N] = feat.T.astype(np.float16)

    # fc_w[f, h*D+d] * attn_r[h, d] summed over d
    wr_fold = np.einsum("fhd,hd->fh", fc_w.reshape(F, H, D), attn_r)
    waug = np.zeros((F, ML), np.float16)
    waug[:, 0:HD] = fc_w.astype(np.float16)
    waug[:, HD:HD + H] = wr_fold.astype(np.float16)

    wlb = np.broadcast_to(
        attn_l.reshape(1, HD).astype(np.float16), (F, HD)).copy()

    consts = np.zeros((128, 160), np.float16)
    consts[:, 0:128] = np.broadcast_to(np.arange(128, dtype=np.float16),
                                       (128, 128))
    consts[:, 128:160] = np.broadcast_to(np.arange(32, dtype=np.float16),
                                         (128, 32))
    return featT, waug, wlb, consts


def kernel(feat, fc_w, attn_l, attn_r, src, dst):
    from concourse.bass_utils import run_bass_kernel_spmd

    featT, waug, wlb, consts = _host_tables(feat, fc_w, attn_l, attn_r)
    cores = _pack(src, dst)

    nc = _get_program()
    in_maps = [
        {"featT": featT, "waug": waug, "wlb": wlb, "consts": consts,
         "idxf": cores[k]["idxf"], "idxe": cores[k]["idxe"],
         "segt": cores[k]["segt"], "dmt": cores[k]["dmt"]}
        for k in range(NCORES)
    ]
    res = run_bass_kernel_spmd(nc, in_maps, core_ids=list(range(NCORES)))
    global LAST_RESULTS
    LAST_RESULTS = res

    outf = np.empty((N, HD), np.float32)
    for k in range(NCORES):
        o = np.asarray(res.results[k]["out"])
        outf[k * NODES_PC:(k + 1) * NODES_PC] = o[:NODES_PC]
    return outf


# revision 17
# speedup vs baseline: 1.0581x; 1.0581x over previous
"""GAT message-passing kernel for Trainium2 (8 NeuronCores, Bass/Tile).

v2 strategy (edge parallelism, batched SWDGE dma_gather):
  - Host: edges sorted by dst; 8 equal node ranges (12500 dst nodes/core).
    Per core, supertile = 128 contiguous dst nodes, capacity 2560 edge slots
    laid out as 4 src-bank segments (bank = src%4) of 640 slots each; pads
    use dummy index 0 with slot-id PAD (one-hot column of zeros).
  - Phase 1 (replicated): T[n] = feat@fc_w (128 fp16 = 256B rows) and
    ERT (er packed 32 nodes/row, 256B) via one matmul pass over feat.
  - Phase 2 per supertile group (G=4): ONE dma_gather per src bank pulls
    all 4 supertiles' feat_src rows (0.34ns/descriptor on Q7 vs 8.5ns for
    per-chunk indirect DMA); one dma_gather pulls packed er rows by dst//32.
    el is computed on-device (fs . attn_l per head); er selected from the
    packed row by a 32-wide one-hot + tree reduce. ex=exp(leaky_relu(el+er));
    scatter-to-slot aggregation via one-hot matmuls accumulating
    [fs*ex | ex] into a [128 slot, 132] PSUM; epilogue divides by the
    summed ex (segment softmax) and streams 128 node rows out per supertile.
"""

import numpy as np

# ---------------- problem constants (hardcoded; kernel.py is self-contained) ---
N = 100000
F = 128
H = 4
D = 32
HD = H * D          # 128
NEG = 0.2
NCORES = 8

NPAD = 100352       # 784 tiles of 128 node rows
NT = NPAD // 128    # 784
NODES_PC = N // NCORES      # 12500 dst nodes per core
NSUP = 98                   # ceil(12500/128) supertiles with real nodes
G = 4                       # supertiles per gather group
NSUP_P = 100                # padded to a multiple of G
NGRP = NSUP_P // G          # 25
W = 2560                    # edge slots per supertile (20 chunks of 128)
NCH = W // 128              # 20
NBANK = 4                   # src banks (src % 4)
Q = W // NBANK              # 640 slots per (supertile, bank)
QCH = Q // 128              # 5 chunks per bank segment
BROWS = NPAD // NBANK       # 25088 rows per T bank
ERT_ROWS = NPAD // 32       # 3136 packed er rows
PAD_SEG = 999.0
ML = HD + H                 # 132 psum cols

WCH = 2048                  # phase-1 feat columns per DMA
G1 = 8                      # node tiles per T write


def _pack(src, dst):
    """Host-side index preprocessing -> per-core static tables."""
    src = np.asarray(src, np.int64)
    dst = np.asarray(dst, np.int64)
    order = np.argsort(dst, kind="stable")
    s_src = src[order]
    s_dst = dst[order]
    cb = np.searchsorted(s_dst, np.arange(NCORES + 1) * NODES_PC)
    cores = []
    for k in range(NCORES):
        e0, e1 = int(cb[k]), int(cb[k + 1])
        ksrc = s_src[e0:e1]
        kdst = s_dst[e0:e1]
        lo = k * NODES_PC
        sup = (kdst - lo) // 128
        bank = ksrc % NBANK
        o2 = np.lexsort((bank, sup))
        ksrc, kdst, sup, bank = ksrc[o2], kdst[o2], sup[o2], bank[o2]
        sb = sup * NBANK + bank
        cnt = np.bincount(sb, minlength=NSUP_P * NBANK)
        assert cnt.max() <= Q, f"bank quota overflow: {cnt.max()} > {Q}"
        seg_start = np.concatenate(([0], np.cumsum(cnt)))[:-1]
        pos_in_seg = np.arange(len(ksrc)) - seg_start[sb]
        w = bank * Q + pos_in_seg            # slot within supertile [0, W)
        lane = w % 128
        ch = w // 128                        # chunk in [0, NCH)

        segt = np.full((128, NSUP_P * NCH), PAD_SEG, np.float16)
        segt[lane, sup * NCH + ch] = (kdst - lo - sup * 128).astype(np.float16)
        dmt = np.zeros((128, NSUP_P * NCH), np.float16)
        dmt[lane, sup * NCH + ch] = (kdst % 32).astype(np.float16)

        # fs gather indices: call (g, b) covers i_call = (sup%G)*Q + pos_in_seg
        idxf = np.zeros((16, NGRP * NBANK * (G * Q) // 16), np.int16)
        call = (sup // G) * NBANK + bank
        i_call = (sup % G) * Q + pos_in_seg
        idxf[i_call % 16, call * (G * Q // 16) + i_call // 16] = \
            (ksrc // NBANK).astype(np.int16)
        # er gather indices: call g covers j_call = (sup%G)*W + w
        idxe = np.zeros((16, NGRP * (G * W) // 16), np.int16)
        j_call = (sup % G) * W + w
        idxe[j_call % 16, (sup // G) * (G * W // 16) + j_call // 16] = \
            (kdst // 32).astype(np.int16)

        cores.append({
            "idxf": np.ascontiguousarray(np.tile(idxf, (8, 1))),
            "idxe": np.ascontiguousarray(np.tile(idxe, (8, 1))),
            "segt": np.ascontiguousarray(segt),
            "dmt": np.ascontiguousarray(dmt),
        })
    return cores


def _patch_queue_aware_lanes():
    """Make tile's DMASW sem-lane assignment SWDGE-queue-aware.

    Stock TileClockTick rotates the 8 DMASW lanes mod 8 in scheduled order,
    oblivious to queue_num; with num_swdge_queues=4 a lane then serves
    multiple queues (HW sem hazard, CoreSim rejects it). Pin queue q to the
    lane pair {q, q+4} by presetting next_sw_dma_idx before each Pool DMA.
    """
    import concourse.tile_sem_assignment as tsa
    import concourse.mybir as mybir
    from concourse.tile_scheduler import DMAInst
    import concourse.bass_isa as bass_isa

    if getattr(tsa, "_gnn_queue_lanes", False):
        return
    tsa._gnn_queue_lanes = True
    orig = tsa.TileClockTick._assign_tick

    def patched(self, inst):
        if (isinstance(inst, DMAInst)
                and not isinstance(inst, bass_isa.UserSyncedRemoteDMADescs)
                and inst.engine == mybir.EngineType.Pool):
            q = getattr(inst, "queue_num", 0) or 0
            tog = getattr(self, "_gnn_qtog", None)
            if tog is None:
                tog = {}
                self._gnn_qtog = tog
            self.next_sw_dma_idx = q + 4 * tog.get(q, 0)
            tog[q] = 1 - tog.get(q, 0)
        return orig(self, inst)

    tsa.TileClockTick._assign_tick = patched


def _build():
    import concourse.bacc as bacc
    import concourse.tile as tile
    import concourse.mybir as mybir

    _patch_queue_aware_lanes()

    F16 = mybir.dt.float16
    F32 = mybir.dt.float32
    I16 = mybir.dt.int16
    AOT = mybir.AluOpType

    nc = bacc.Bacc("TRN2", target_bir_lowering=False, debug=False,
                   num_swdge_queues=4)
    featT = nc.dram_tensor("featT", [F, NPAD], F16, kind="ExternalInput")
    waug = nc.dram_tensor("waug", [F, ML], F16, kind="ExternalInput")
    wlb = nc.dram_tensor("wlb", [F, HD], F16, kind="ExternalInput")
    consts = nc.dram_tensor("consts", [128, 160], F16, kind="ExternalInput")
    idxf_d = nc.dram_tensor("idxf", [128, NGRP * NBANK * G * Q // 16], I16,
                            kind="ExternalInput")
    idxe_d = nc.dram_tensor("idxe", [128, NGRP * G * W // 16], I16,
                            kind="ExternalInput")
    segt_d = nc.dram_tensor("segt", [128, NSUP_P * NCH], F16,
                            kind="ExternalInput")
    dmt_d = nc.dram_tensor("dmt", [128, NSUP_P * NCH], F16,
                           kind="ExternalInput")
    T = nc.dram_tensor("T", [BROWS, NBANK * 128], F16, kind="Internal")
    ERT = nc.dram_tensor("ERT", [ERT_ROWS, 128], F16, kind="Internal")
    out = nc.dram_tensor("out", [NSUP_P * 128, HD], F32, kind="ExternalOutput")

    with tile.TileContext(nc) as tc:
        with tc.tile_pool(name="const", bufs=1) as const:
            waug_sb = const.tile([F, ML], F16)
            nc.sync.dma_start(out=waug_sb[:], in_=waug[:, :])
            wlb_sb = const.tile([F, HD], F16)
            nc.sync.dma_start(out=wlb_sb[:], in_=wlb[:, :])
            cst = const.tile([128, 160], F16)
            nc.sync.dma_start(out=cst[:], in_=consts[:, :])
            iota128 = cst[:, 0:128]
            iota32 = cst[:, 128:160]
            er_sb = const.tile([128, NT * H], F16)

            # ---- phase 1: T = feat @ fc_w (fp16), ERT = packed er ----
            with tc.tile_pool(name="fp", bufs=3) as fpool, \
                 tc.tile_pool(name="p1ps", bufs=8, space="PSUM") as p1ps, \
                 tc.tile_pool(name="st1", bufs=4) as st1p:
                tpw = WCH // 128
                for wb in range(NPAD // WCH):
                    fsb = fpool.tile([F, WCH], F16)
                    nc.sync.dma_start(
                        out=fsb[:], in_=featT[:, wb * WCH:(wb + 1) * WCH])
                    for grp in range(tpw // G1):
                        stg = st1p.tile([F, G1 * ML], F16)
                        sv = stg[:].rearrange("p (j c) -> p j c", j=G1)
                        for j in range(G1):
                            ps = p1ps.tile([128, ML], F32)
                            col0 = (grp * G1 + j) * 128
                            nc.tensor.matmul(
                                out=ps[:],
                                lhsT=fsb[:, col0:col0 + 128],
                                rhs=waug_sb[:],
                                start=True, stop=True,
                            )
                            nc.vector.tensor_copy(
                                out=sv[:, j, :], in_=ps[:])
                        t0 = wb * tpw + grp * G1
                        nc.vector.tensor_copy(
                            out=er_sb[:].rearrange(
                                "p (t h) -> p t h", h=H)[:, t0:t0 + G1, :],
                            in_=sv[:, :, HD:HD + H])
                        # node n=(t0+j)*128+p -> T row n//4, col (n%4)*128+c
                        nc.sync.dma_start(
                            out=T[t0 * 32:(t0 + G1) * 32, :].rearrange(
                                "(j q) (b c) -> (q b) j c", q=32, c=128),
                            in_=sv[:, :, 0:HD],
                        )
                # node n=t*128+p -> ERT row n//32, col (n%32)*4+h
                nc.sync.dma_start(
                    out=ERT[:, :].rearrange("(t a) (b h) -> (a b) t h",
                                            a=4, h=H),
                    in_=er_sb[:].rearrange("p (t h) -> p t h", h=H),
                )

            # ---- phase 2 ----
            with tc.tile_pool(name="ix", bufs=2) as ixp, \
                 tc.tile_pool(name="sg", bufs=2) as sgp, \
                 tc.tile_pool(name="fsg", bufs=2) as fsgp, \
                 tc.tile_pool(name="erg", bufs=2) as ergp, \
                 tc.tile_pool(name="pt", bufs=2) as ptp, \
                 tc.tile_pool(name="sc", bufs=2) as scp, \
                 tc.tile_pool(name="mx", bufs=3) as mxp, \
                 tc.tile_pool(name="ot", bufs=3) as otp, \
                 tc.tile_pool(name="p2ps", bufs=4, space="PSUM") as p2ps:
                qrr = [0]
                for g in range(NGRP):
                    idxf_sb = ixp.tile([128, NBANK * G * Q // 16], I16, tag="f")
                    nc.sync.dma_start(
                        out=idxf_sb[:],
                        in_=idxf_d[:, g * NBANK * G * Q // 16:
                                   (g + 1) * NBANK * G * Q // 16])
                    idxe_sb = ixp.tile([128, G * W // 16], I16, tag="e")
                    nc.sync.dma_start(
                        out=idxe_sb[:],
                        in_=idxe_d[:, g * G * W // 16:(g + 1) * G * W // 16])
                    seg_sb = sgp.tile([128, G * NCH], F16, tag="s")
                    nc.sync.dma_start(
                        out=seg_sb[:],
                        in_=segt_d[:, g * G * NCH:(g + 1) * G * NCH])
                    dm_sb = sgp.tile([128, G * NCH], F16, tag="d")
                    nc.sync.dma_start(
                        out=dm_sb[:],
                        in_=dmt_d[:, g * G * NCH:(g + 1) * G * NCH])

                    # ucode caps one call at 1024 descriptors; round-robin the
                    # 4 SWDGE queues (desc-gen contexts run concurrently)
                    def _gather(out_ap, in_ap, idxs_ap, nidx, estep):
                        nc.gpsimd.dma_gather(
                            out_ap=out_ap, in_ap=in_ap, idxs_ap=idxs_ap,
                            num_idxs=nidx, num_idxs_reg=nidx,
                            elem_size=128, elem_step=estep,
                            queue_num=qrr[0])
                        qrr[0] = (qrr[0] + 1) % 4

                    fsg = fsgp.tile([128, NBANK * G * Q], F16)
                    for b in range(NBANK):
                        for p0 in range(0, G * Q, 1024):
                            n = min(1024, G * Q - p0)
                            a0 = b * G * Q + p0
                            _gather(
                                fsg[:, a0:a0 + n]
                                    .rearrange("p (k e) -> p k e", e=128),
                                T[:, b * 128:(b + 1) * 128],
                                idxf_sb[:, a0 // 16:(a0 + n) // 16],
                                n, NBANK * 128)
                    erg = ergp.tile([128, G * W], F16)
                    for p0 in range(0, G * W, 1024):
                        n = min(1024, G * W - p0)
                        _gather(
                            erg[:, p0:p0 + n]
                                .rearrange("p (k e) -> p k e", e=128),
                            ERT[:, :],
                            idxe_sb[:, p0 // 16:(p0 + n) // 16],
                            n, 128)

                    for s4 in range(G):
                        sup = g * G + s4
                        # fs view: [128, bank, chunk-in-bank, 128]
                        fs_s = fsg[:].rearrange(
                            "p (b s j e) -> p b s j e", b=NBANK, s=G, e=128
                        )[:, :, s4]
                        seg_s = seg_sb[:, s4 * NCH:(s4 + 1) * NCH]
                        dm_s = dm_sb[:, s4 * NCH:(s4 + 1) * NCH]
                        erv = erg[:].rearrange(
                            "p (s k m h) -> p s k m h", s=G, m=32, h=H
                        )[:, s4]

                        # one-hot edge->slot (P_t) for aggregation
                        ptt = ptp.tile([128, NCH * 128], F16)
                        nc.vector.tensor_tensor(
                            out=ptt[:].rearrange("p (c s) -> p c s", s=128),
                            in0=seg_s[:, :, None].broadcast_to([128, NCH, 128]),
                            in1=iota128[:, None, :].broadcast_to([128, NCH, 128]),
                            op=AOT.is_equal,
                        )

                        # el = fs . attn_l per head
                        tmp = mxp.tile([128, W], F16, tag="tmp")
                        nc.vector.tensor_tensor(
                            out=tmp[:].rearrange("p (b j e) -> p b j e",
                                                 b=NBANK, e=128),
                            in0=fs_s,
                            in1=wlb_sb[:, None, None, :].broadcast_to(
                                [128, NBANK, QCH, 128]),
                            op=AOT.mult,
                        )
                        el = scp.tile([128, NCH * H], F32, tag="el")
                        nc.vector.tensor_reduce(
                            el[:].rearrange("p (c h) -> p c h", h=H),
                            tmp[:].rearrange("p (c h d) -> p c h d", h=H, d=D),
                            mybir.AxisListType.X,
                            AOT.add,
                        )

                        # er select: one-hot over 32-node pack + tree reduce
                        oh = scp.tile([128, NCH * 32], F16, tag="oh")
                        nc.vector.tensor_tensor(
                            out=oh[:].rearrange("p (c m) -> p c m", m=32),
                            in0=dm_s[:, :, None].broadcast_to([128, NCH, 32]),
                            in1=iota32[:, None, :].broadcast_to([128, NCH, 32]),
                            op=AOT.is_equal,
                        )
                        t0 = scp.tile([128, NCH * 128], F16, tag="t0")
                        t0v = t0[:].rearrange("p (c m h) -> p c m h", m=32, h=H)
                        nc.vector.tensor_tensor(
                            out=t0v,
                            in0=erv,
                            in1=oh[:].rearrange("p (c m) -> p c m", m=32)
                                [:, :, :, None].broadcast_to([128, NCH, 32, H]),
                            op=AOT.mult,
                        )
                        m = 32
                        cur = t0v
                        cur_tile = t0
                        while m > 1:
                            m //= 2
                            nxt = scp.tile([128, NCH * m * H], F16,
                                           tag=f"t{m}")
                            nv = nxt[:].rearrange("p (c m h) -> p c m h",
                                                  m=m, h=H)
                            nc.vector.tensor_tensor(
                                out=nv, in0=cur[:, :, 0:m], in1=cur[:, :, m:2 * m],
                                op=AOT.add)
                            cur = nv
                            cur_tile = nxt
                        # u = el + er ; leaky relu ; exp
                        u = scp.tile([128, NCH * H], F32, tag="u")
                        nc.vector.tensor_tensor(
                            out=u[:], in0=el[:], in1=cur_tile[:],
                            op=AOT.add)
                        u2 = scp.tile([128, NCH * H], F32, tag="u2")
                        nc.vector.scalar_tensor_tensor(
                            out=u2[:], in0=u[:], scalar=NEG, in1=u[:],
                            op0=AOT.mult, op1=AOT.max)
                        ex = scp.tile([128, NCH * H], F32, tag="ex")
                        nc.scalar.activation(
                            out=ex[:], in_=u2[:],
                            func=mybir.ActivationFunctionType.Exp)
                        exv = ex[:].rearrange("p (c h) -> p c h", h=H)

                        # mx = [fs * ex | ex]
                        mx = mxp.tile([128, NCH * ML], F16, tag="mx")
                        mv = mx[:].rearrange("p (c w) -> p c w", w=ML)
                        mv4 = mx[:].rearrange("p (b j w) -> p b j w",
                                              b=NBANK, w=ML)
                        ex4 = ex[:].rearrange("p (b j h) -> p b j h",
                                              b=NBANK, h=H)
                        nc.vector.tensor_copy(out=mv[:, :, HD:HD + H], in_=exv)
                        for h in range(H):
                            nc.vector.tensor_tensor(
                                out=mv4[:, :, :, h * D:(h + 1) * D],
                                in0=fs_s[:, :, :, h * D:(h + 1) * D],
                                in1=ex4[:, :, :, h:h + 1].broadcast_to(
                                    [128, NBANK, QCH, D]),
                                op=AOT.mult,
                            )

                        # aggregation
                        ps = p2ps.tile([128, ML], F32)
                        for c in range(NCH):
                            nc.tensor.matmul(
                                out=ps[:],
                                lhsT=ptt[:, c * 128:(c + 1) * 128],
                                rhs=mx[:, c * ML:(c + 1) * ML],
                                start=(c == 0), stop=(c == NCH - 1),
                            )
                        r0 = otp.tile([128, H], F32, tag="r0")
                        nc.vector.tensor_scalar_max(r0[:], ps[:, HD:HD + H],
                                                    1e-30)
                        r1 = otp.tile([128, H], F32, tag="r1")
                        nc.vector.reciprocal(r1[:], r0[:])
                        ot = otp.tile([128, HD], F32, tag="ot")
                        nc.vector.tensor_tensor(
                            out=ot[:].rearrange("p (h d) -> p h d", h=H),
                            in0=ps[:, 0:HD].rearrange("p (h d) -> p h d", h=H),
                            in1=r1[:, :, None].broadcast_to([128, H, D]),
                            op=AOT.mult,
                        )
                        nc.sync.dma_start(
                            out=out[sup * 128:(sup + 1) * 128, :],
                            in_=ot[:],
                        )

    nc.compile()
    return nc


_NC_CACHE = {}
LAST_RESULTS = None


def _get_program():
    if "v2" not in _NC_CACHE:
        _NC_CACHE["v2"] = _build()
    return _NC_CACHE["v2"]


def _host_tables(feat, fc_w, attn_l, attn_r):
    feat = np.asarray(feat, np.float32)
    fc_w = np.asarray(fc_w, np.float64)
    attn_l = np.asarray(attn_l, np.float64).reshape(H, D)
    attn_r = np.asarray(attn_r, np.float64).reshape(H, D)

    featT = np.zeros((F, NPAD), np.float16)
    featT[:, :N] = feat.T.astype(np.float16)

    # fc_w[f, h*D+d] * attn_r[h, d] summed over d
    wr_fold = np.einsum("fhd,hd->fh", fc_w.reshape(F, H, D), attn_r)
    waug = np.zeros((F, ML), np.float16)
    waug[:, 0:HD] = fc_w.astype(np.float16)
    waug[:, HD:HD + H] = wr_fold.astype(np.float16)

    wlb = np.broadcast_to(
        attn_l.reshape(1, HD).astype(np.float16), (F, HD)).copy()

    consts = np.zeros((128, 160), np.float16)
    consts[:, 0:128] = np.broadcast_to(np.arange(128, dtype=np.float16),
                                       (128, 128))
    consts[:, 128:160] = np.broadcast_to(np.arange(32, dtype=np.float16),
                                         (128, 32))
    return featT, waug, wlb, consts


def kernel(feat, fc_w, attn_l, attn_r, src, dst):
    from concourse.bass_utils import run_bass_kernel_spmd

    featT, waug, wlb, consts = _host_tables(feat, fc_w, attn_l, attn_r)
    cores = _pack(src, dst)

    nc = _get_program()
    in_maps = [
        {"featT": featT, "waug": waug, "wlb": wlb, "consts": consts,
         "idxf": cores[k]["idxf"], "idxe": cores[k]["idxe"],
         "segt": cores[k]["segt"], "dmt": cores[k]["dmt"]}
        for k in range(NCORES)
    ]
    res = run_bass_kernel_spmd(nc, in_maps, core_ids=list(range(NCORES)))
    global LAST_RESULTS
    LAST_RESULTS = res

    outf = np.empty((N, HD), np.float32)
    for k in range(NCORES):
        o = np.asarray(res.results[k]["out"])
        outf[k * NODES_PC:(k + 1) * NODES_PC] = o[:NODES_PC]
    return outf
